# revision 39
# baseline (speedup 1.0000x reference)
"""Trainium2 Bass kernel for nn_ExpSelfAttention (dense transformer block).

Math (per batch item b):
    y  = LN(x; g1, beta1);  z = y @ w_lin.T + b_lin
    attn = W @ z            (W = causal exp-decay matrix, alpha=0.9)
    x2 = x + attn
    y2 = LN(x2; g2, beta2); h = relu(y2 @ w1.T + b1)
    out = x2 + h @ w2.T + b2

Sharding: data parallel over batch (16 / 8 cores = 2 per core); weights and
the (input-independent) decay-matrix blocks replicated. No collectives.

Kernel strategy per core (v3 - mixed precision, skewed pipeline):
  - Attention path in bf16 (x itself is loaded as bf16 by a casting DMA;
    proj + banded mixing matmuls and PE transposes at 1 cyc/row). x2 kept
    in bf16; all LN statistics taken on-chip in f32 via bn_stats.
  - b_lin folded in via x_pzb = x + zb on Pool (W rows sum to 1, so
    W@(z+zb) = W@z + zb); b2 pre-added on Pool (x2b) off the critical path.
  - FFN matmuls in fp8e4 (e4m3) DoubleRow: 256 contraction rows per
    instruction at 0.5 cyc per output element (4x less PE time than f32).
    Weights pre-scaled x16 (w1) / x32 (w2) on the host to center fp8's
    [2^-6, 240] range; the 512x factor is unwound in the output eviction.
  - The S x S decay matmul is block-banded (alpha^128 ~ 1.4e-6): exact
    block-diag + 1 lag matmul per 128-token block.
  - Transpose PSUM->SBUF evictions ride HWDGE DMA (same-dtype copies);
    LN normalizes use DVE 2x/4x SBUF fast modes; relu evictions on ACT
    with per-f-tile bias; sqrt batched in pairs on ACT (one act table).
  - FFN is software-skewed one step behind the attention pipeline so the
    LN2 stats chain of step i hides under FFN matmuls of step i-1.
"""

import sys
from contextlib import ExitStack

for _p in ("/opt/trn_rl_repo", "/opt/pypackages"):
    if _p not in sys.path:
        sys.path.insert(0, _p)

import numpy as np
import ml_dtypes

import concourse.bass as bass
import concourse.mybir as mybir
import concourse.tile as tile
from concourse import bacc
from concourse.bass_utils import run_bass_kernel_spmd
from concourse.masks import make_identity

ALPHA, EPS = 0.9, 1e-5
S, B, D, FF = 2048, 16, 512, 2048
NCORES = 8
BL = B // NCORES            # batch items per core
T = 128                     # mixing block
CB = 4                      # blocks per chunk (step = 512 tokens)
NBLK = S // T               # 16
NCHUNK = NBLK // CB         # 4
NFT = FF // 128             # 16 f-tiles
KD = D // 128               # 4 d-tiles
NLAG = 1                    # decay lag blocks kept (lag>=2 < 2e-12 relative)
W1SC, W2SC = 16.0, 32.0     # fp8 weight pre-scales
OSC = 1.0 / (W1SC * W2SC)   # output unscale
RELU_DVE = (3, 6, 9, 12, 15)  # f-tiles whose relu evict runs on DVE

F32 = mybir.dt.float32
BF16 = mybir.dt.bfloat16
F8 = mybir.dt.float8e4
AF = mybir.ActivationFunctionType
ALU = mybir.AluOpType
DR = mybir.MatmulPerfMode.DoubleRow

NPBF16 = ml_dtypes.bfloat16
NPF8 = mybir.dt.np(F8)      # ml_dtypes.float8_e4m3 (max 240)


def _host_consts():
    """Decay-matrix derived constants, f64 -> f32 (mirrors reference)."""
    i = np.arange(S, dtype=np.float64)
    diff = i[:, None] - i[None, :]
    with np.errstate(under="ignore"):
        W = np.where(diff >= 0, ALPHA ** (diff + 1), 0.0)
        W = W + np.diag(1.0 - W.sum(axis=1))
        W = W.astype(np.float32)
        blocks = [
            np.ascontiguousarray(W[c * T : (c + 1) * T, c * T : (c + 1) * T].T)
            for c in range(NBLK)
        ]
        uniq, idx = [], []
        for blk in blocks:
            for j, u in enumerate(uniq):
                if np.array_equal(blk, u):
                    idx.append(j)
                    break
            else:
                idx.append(len(uniq))
                uniq.append(blk)
        wblkT = np.stack(uniq)  # [NU, T, T]
        lags = []
        for l in range(1, NLAG + 1):
            L = W[l * T : (l + 1) * T, 0:T]
            lags.append(np.ascontiguousarray(L.T))
        wlagT = np.stack(lags)  # [NLAG, T, T]
    return wblkT.astype(np.float32), idx, wlagT.astype(np.float32)


_WBLKT, _BLKIDX, _WLAGT = _host_consts()
NU = _WBLKT.shape[0]

_NC_CACHE = {}


def build_nc():
    key = "v3"
    if key in _NC_CACHE:
        return _NC_CACHE[key]
    nc = bacc.Bacc()

    x_d = nc.declare_dram_parameter("x", [S, BL, D], F32, isOutput=False)
    wp_d = nc.declare_dram_parameter("wp", [D, D], BF16, isOutput=False)
    zb_d = nc.declare_dram_parameter("zb", [D], BF16, isOutput=False)
    w1_d = nc.declare_dram_parameter("w1t8", [D, FF], F8, isOutput=False)
    hb_d = nc.declare_dram_parameter("hb16", [FF], F32, isOutput=False)
    w2_d = nc.declare_dram_parameter("w2t8", [FF, D], F8, isOutput=False)
    b2_d = nc.declare_dram_parameter("b2", [D], F32, isOutput=False)
    wblk_d = nc.declare_dram_parameter("wblk", [NU, T, T], BF16, isOutput=False)
    wlag_d = nc.declare_dram_parameter("wlag", [NLAG, T, T], BF16, isOutput=False)
    out_d = nc.declare_dram_parameter("out", [S, BL, D], F32, isOutput=True)

    with tile.TileContext(nc) as tc, ExitStack() as ctx:
        pool = lambda name, bufs, **kw: ctx.enter_context(
            tc.tile_pool(name=name, bufs=bufs, **kw)
        )
        wgt = pool("wgt", 1)
        stage = pool("stage", 1)
        xin = pool("xin", 18)
        lnp = pool("ln", 4)
        yppp = pool("ypp", 3)
        xtp = pool("xt", 4)
        y2tp = pool("y2t", 6)
        zp = pool("z", 16)
        x2p = pool("x2", 9)
        x2bp = pool("x2b", 9)
        hp = pool("h", 2)
        outp = pool("outp", 3)
        psmm = pool("psmm", 5, space="PSUM")
        pstr = pool("pstr", 3, space="PSUM")

        # ---------------- one-time setup ----------------
        steps = [(b, c) for b in range(BL) for c in range(NCHUNK)]
        xpre = {}

        def preload_x(i):
            if i >= len(steps) or i in xpre:
                return
            b, c = steps[i]
            tiles = []
            for t in range(CB):
                s0 = (c * CB + t) * T
                xt = xin.tile([128, D], F32, tag="x")
                nc.sync.dma_start(xt[:], x_d.ap()[s0 : s0 + T, b, :])
                tiles.append(xt)
            xpre[i] = tiles

        ident_f = stage.tile([128, 128], F32, tag="ident_f")
        make_identity(nc, ident_f[:])
        ident_bf = wgt.tile([128, 128], BF16, tag="ident_bf")
        nc.vector.tensor_copy(ident_bf[:], ident_f[:])
        ident_f8 = wgt.tile([128, 128], F8, tag="ident_f8")
        nc.vector.tensor_copy(ident_f8[:], ident_f[:])
        eps_t = wgt.tile([128, 1], F32, tag="eps")
        nc.vector.memset(eps_t[:], EPS)
        ones_r = wgt.tile([1, 128], BF16, tag="ones_r")
        nc.vector.memset(ones_r[:], 1.0)
        zb_r = wgt.tile([1, D], BF16, tag="zb_r")
        nc.sync.dma_start(zb_r[:], bass.AP(tensor=zb_d, offset=0, ap=[[0, 1], [1, D]]))
        b2_bc = wgt.tile([128, D], F32, tag="b2")
        nc.sync.dma_start(
            b2_bc[:], bass.AP(tensor=b2_d, offset=0, ap=[[0, 128], [1, D]])
        )
        hb_sb = wgt.tile([128, NFT], F32, tag="hb")
        nc.sync.dma_start(
            hb_sb[:], bass.AP(tensor=hb_d, offset=0, ap=[[1, 128], [128, NFT]])
        )
        wp_r = wgt.tile([128, KD, D], BF16, tag="wp")
        nc.sync.dma_start(wp_r[:], wp_d.ap().rearrange("(kd p) e -> p kd e", p=128))
        wblk_r = wgt.tile([128, NU, T], BF16, tag="wblk")
        nc.sync.dma_start(wblk_r[:], wblk_d.ap().rearrange("b j r -> j b r"))
        wlag_r = wgt.tile([128, NLAG, T], BF16, tag="wlag")
        nc.sync.dma_start(wlag_r[:], wlag_d.ap().rearrange("b j r -> j b r"))

        preload_x(0)
        preload_x(1)

        # ---------------- helpers ----------------
        def stats_one(xt, mvs, t):
            st = lnp.tile([128, 6], F32, tag="st")
            nc.vector.bn_stats(st[:], xt[:])
            nc.vector.bn_aggr(mvs[:, t, :], st[:])

        def sqrt_pair(mvs, sq4, t0):
            nc.scalar.activation(
                sq4[:, t0 : t0 + 2],
                mvs[:, t0 : t0 + 2, 1:2].rearrange("p a b -> p (a b)"),
                AF.Sqrt, bias=eps_t[:], scale=1.0,
            )

        def norm_transp_pair(x01, mvs, sq4, t0, odt, ident, dest_ap):
            """Normalize+transpose two token tiles; one paired PSUM evict.

            dest_ap must enumerate (tp, kd, tok-128) after the partition dim."""
            pt = pstr.tile([128, 2, 512], odt, tag="tr")
            for tp in range(2):
                t = t0 + tp
                ypp = yppp.tile([128, D], odt, tag="ypp")
                nc.gpsimd.tensor_scalar(
                    out=ypp[:], in0=x01[tp][:],
                    scalar1=mvs[:, t, 0:1], scalar2=sq4[:, t : t + 1],
                    op0=ALU.subtract, op1=ALU.divide,
                )
                for kd in range(KD):
                    nc.tensor.transpose(
                        pt[:, tp, kd * 128 : (kd + 1) * 128],
                        ypp[:, kd * 128 : (kd + 1) * 128],
                        ident[:],
                    )
            nc.scalar.activation(
                dest_ap,
                pt[:].rearrange("p a (c b) -> p a c b", b=128),
                AF.Copy,
            )

        # ---------------- pipeline stages ----------------
        zall = {b: [] for b in range(BL)}
        a_out = {}

        st1 = {}

        def stage_a_stats(i):
            """LN1 statistics for step i (run two steps ahead)."""
            if i >= len(steps):
                return
            xts = xpre[i]
            mvs = lnp.tile([128, CB, 2], F32, tag="l1mv")
            sq4 = lnp.tile([128, CB], F32, tag="l1sq")
            for t in range(CB):
                stats_one(xts[t], mvs, t)
                if t % 2 == 1:
                    sqrt_pair(mvs, sq4, t - 1)
            st1[i] = (mvs, sq4)

        def stage_a_norm(i):
            """LN1 normalize + transpose for step i -> xT pair tiles."""
            if i >= len(steps):
                return
            xts = xpre[i]
            mvs, sq4 = st1.pop(i)
            xT = []
            for t0 in (0, 2):
                xTp = xtp.tile([128, 2, KD, 128], BF16, tag="xT")
                norm_transp_pair(xts[t0 : t0 + 2], mvs, sq4, t0, BF16,
                                 ident_bf, xTp[:])
                xT.append(xTp)
            a_out[i] = xT

        def stage_b(i):
            """proj matmul + z eviction for step i."""
            if i >= len(steps):
                return
            b, c = steps[i]
            xT = a_out.pop(i)
            for t in range(CB):
                pz = psmm.tile([128, D], F32, tag="mm")
                for kd in range(KD):
                    nc.tensor.matmul(
                        pz[:], xT[t // 2][:, t % 2, kd, :], wp_r[:, kd, :],
                        start=(kd == 0), stop=(kd == KD - 1),
                    )
                zt = zp.tile([128, D], BF16, tag="z")
                nc.scalar.activation(zt[:], pz[:], AF.Copy)
                zall[b].append(zt)

        h_cur = {}

        def ffn1(i):
            """FFN1 matmuls + relu evictions for step i (one step behind)."""
            if i < 0:
                return
            y2T, _ = f_in[i]
            h8 = hp.tile([128, NFT // 2, 2, 512], F8, tag="h")
            for ft in range(NFT):
                ph = psmm.tile([128, 512], F32, tag="mm")
                for pr in range(2):
                    for kd2 in range(2):
                        nc.tensor.matmul(
                            ph[:, pr * 256 : (pr + 1) * 256],
                            w18_r[:, kd2, :, ft * 128 : (ft + 1) * 128],
                            y2T[pr][:, 2 * kd2 : 2 * kd2 + 2, :, :],
                            start=(kd2 == 0), stop=(kd2 == 1),
                            perf_mode=DR,
                        )
                hdst = h8[:, ft // 2, ft % 2, :]
                hbc = hb_sb[:, ft : ft + 1]
                if ft in RELU_DVE:
                    nc.vector.tensor_scalar(
                        out=hdst, in0=ph[:], scalar1=hbc, scalar2=0.0,
                        op0=ALU.add, op1=ALU.max,
                    )
                else:
                    nc.scalar.activation(hdst, ph[:], AF.Relu, bias=hbc, scale=1.0)
            h_cur[i] = h8

        def ffn2(i):
            """FFN2 matmuls + output eviction/DMA for step i."""
            if i < 0:
                return
            b, c = steps[i]
            _, x2bts = f_in.pop(i)
            h8 = h_cur.pop(i)
            for t in range(CB):
                s0 = (c * CB + t) * T
                po = psmm.tile([128, D], F32, tag="mm")
                for eh in range(2):
                    for kt in range(NFT // 2):
                        nc.tensor.matmul(
                            po[:, eh * 256 : (eh + 1) * 256],
                            h8[:, kt, :, t * 128 : (t + 1) * 128],
                            w28_r[:, kt, :, eh * 256 : (eh + 1) * 256],
                            start=(kt == 0), stop=(kt == NFT // 2 - 1),
                            perf_mode=DR,
                        )
                ot = outp.tile([128, D], F32, tag="o")
                nc.vector.scalar_tensor_tensor(
                    out=ot[:], in0=po[:], scalar=OSC, in1=x2bts[t][:],
                    op0=ALU.mult, op1=ALU.add,
                )
                nc.sync.dma_start(out_d.ap()[s0 : s0 + T, b, :], ot[:])

        # prologue: steps 0 and 1 go through the full LN1+proj+z chain so
        # the main loop (which produces z two steps ahead) can start at i=0
        preload_x(2)
        stage_a_stats(0)
        stage_a_stats(1)
        stage_a_norm(0)
        stage_b(0)
        stage_a_norm(1)
        stage_b(1)

        # big fp8 FFN weights, chunked between early pipeline stages
        w18_r = wgt.tile([128, 2, 2, FF], F8, tag="w18")
        w28_r = wgt.tile([128, NFT // 2, 2, D], F8, tag="w28")
        w18_ap = w1_d.ap().rearrange("(kd2 i p) f -> p kd2 i f", p=128, i=2)
        w28_ap = w2_d.ap().rearrange("(kt i p) e -> p kt i e", p=128, i=2)
        wload = [
            lambda kd2=kd2: nc.sync.dma_start(
                w18_r[:, kd2, :, :], w18_ap[:, kd2, :, :]
            )
            for kd2 in range(2)
        ] + [
            lambda k4=k4: nc.sync.dma_start(
                w28_r[:, 2 * k4 : 2 * k4 + 2, :, :],
                w28_ap[:, 2 * k4 : 2 * k4 + 2, :, :],
            )
            for k4 in range(4)
        ]
        wload.reverse()
        if wload:
            wload.pop()()

        f_in = {}
        for i, (b, c) in enumerate(steps):
            preload_x(i + 3)
            xts = xpre.pop(i)
            # --- FFN1 of the previous step first: PE work that is ready
            # immediately, and its relu evictions drain early ---
            ffn1(i - 1)
            # --- mixing (+ zb ones-row bias matmul) + x2 + LN2 stats ---
            mvs2 = lnp.tile([128, CB, 2], F32, tag="l2mv")
            sq42 = lnp.tile([128, CB], F32, tag="l2sq")
            x2ts = []
            for t in range(CB):
                blk = c * CB + t
                nmix = 1 + min(blk, NLAG)
                pm = psmm.tile([128, D], F32, tag="mm")
                for l in range(nmix):
                    lhs = (
                        wblk_r[:, _BLKIDX[blk], :] if l == 0
                        else wlag_r[:, l - 1, :]
                    )
                    nc.tensor.matmul(
                        pm[:], lhs, zall[b][blk - l][:],
                        start=(l == 0), stop=False,
                    )
                nc.tensor.matmul(pm[:], ones_r[:], zb_r[:], start=False, stop=True)
                x2t = x2p.tile([128, D], BF16, tag="x2")
                nc.vector.tensor_add(x2t[:], pm[:], xts[t][:])
                x2ts.append(x2t)
                stats_one(x2t, mvs2, t)
                nc.scalar.activation(
                    sq42[:, t : t + 1], mvs2[:, t, 1:2],
                    AF.Sqrt, bias=eps_t[:], scale=1.0,
                )
            ffn2(i - 1)
            for _ in range(min(2, len(wload))):
                wload.pop()()
            # --- LN2 normalize + transpose -> y2T pair tiles (fp8).
            # Tile layout [128, KD, tp, 128] (kd-major) so an FFN1 DoubleRow
            # rhs slice [:, 2kd2:2kd2+2, :, :] merges (tp, tok) into N=256. ---
            y2T = []
            for t0 in (0, 2):
                y2Tp = y2tp.tile([128, KD, 2, 128], F8, tag="y2T")
                norm_transp_pair(x2ts[t0 : t0 + 2], mvs2, sq42, t0, F8,
                                 ident_f8,
                                 y2Tp[:].rearrange("p kd tp b -> p tp kd b"))
                y2T.append(y2Tp)
            # --- step i+2's LN1 chain, proj + z eviction: a full iteration
            # of slack before mixing(i+2) consumes z ---
            stage_a_stats(i + 2)
            x2bts = []
            for t in range(CB):
                x2bt = x2bp.tile([128, D], F32, tag="x2b")
                nc.gpsimd.tensor_add(x2bt[:], x2ts[t][:], b2_bc[:])
                x2bts.append(x2bt)
            f_in[i] = (y2T, x2bts)
            for _ in range(min(3, len(wload))):
                wload.pop()()
            stage_a_norm(i + 2)
            stage_b(i + 2)
        ffn1(len(steps) - 1)
        ffn2(len(steps) - 1)

    nc.compile()
    _NC_CACHE[key] = nc
    return nc


def _prep_inputs(x, w_lin, b_lin, w1, b1, w2, b2, g1, beta1, g2, beta2):
    f32 = np.float32
    wp = np.ascontiguousarray(w_lin.T * g1[:, None]).astype(NPBF16)
    zb = (w_lin.astype(np.float64) @ beta1.astype(np.float64) + b_lin).astype(
        f32
    ).astype(NPBF16)
    w1t8 = np.ascontiguousarray(W1SC * w1.T * g2[:, None]).astype(NPF8)
    hb16 = (W1SC * (w1.astype(np.float64) @ beta2.astype(np.float64) + b1)).astype(f32)
    w2t8 = np.ascontiguousarray(W2SC * w2.T).astype(NPF8)
    shared = {
        "wp": wp,
        "zb": zb,
        "w1t8": w1t8,
        "hb16": hb16,
        "w2t8": w2t8,
        "b2": b2.astype(f32),
        "wblk": _WBLKT.astype(NPBF16),
        "wlag": _WLAGT.astype(NPBF16),
    }
    in_maps = []
    for cc in range(NCORES):
        m = dict(shared)
        m["x"] = np.ascontiguousarray(x[:, cc * BL : (cc + 1) * BL, :]).astype(f32)
        in_maps.append(m)
    return in_maps


def kernel(**inputs):
    nc = build_nc()
    in_maps = _prep_inputs(**inputs)
    res = run_bass_kernel_spmd(nc, in_maps, list(range(NCORES)))
    out = np.concatenate([r["out"] for r in res.results], axis=1)
    return out.astype(np.float32)


if __name__ == "__main__":
    rng = np.random.default_rng(0)
    demo = {
        "x": rng.standard_normal((S, B, D)).astype(np.float32),
        "w_lin": rng.standard_normal((D, D)).astype(np.float32) * D**-0.5,
        "b_lin": rng.standard_normal((D,)).astype(np.float32) * 0.01,
        "w1": rng.standard_normal((FF, D)).astype(np.float32) * D**-0.5,
        "b1": rng.standard_normal((FF,)).astype(np.float32) * 0.01,
        "w2": rng.standard_normal((D, FF)).astype(np.float32) * FF**-0.5,
        "b2": rng.standard_normal((D,)).astype(np.float32) * 0.01,
        "g1": np.ones(D, np.float32),
        "beta1": np.zeros(D, np.float32),
        "g2": np.ones(D, np.float32),
        "beta2": np.zeros(D, np.float32),
    }
    out = kernel(**demo)
    print("ok", out.shape, out.dtype)


# revision 41
# speedup vs baseline: 1.0649x; 1.0649x over previous
"""Trainium2 Bass kernel for nn_ExpSelfAttention (dense transformer block).

Math (per batch item b):
    y  = LN(x; g1, beta1);  z = y @ w_lin.T + b_lin
    attn = W @ z            (W = causal exp-decay matrix, alpha=0.9)
    x2 = x + attn
    y2 = LN(x2; g2, beta2); h = relu(y2 @ w1.T + b1)
    out = x2 + h @ w2.T + b2

Sharding: data parallel over batch (16 / 8 cores = 2 per core); weights and
the (input-independent) decay-matrix blocks replicated. No collectives.

Kernel strategy per core (mixed precision):
  - Attention path in bf16 (proj + banded mixing matmuls, PE transposes at
    1 cyc/row); residuals and LN stats in f32. b_lin folded into the mixing
    PSUM via a K=1 ones-row bias matmul (W rows sum to 1, so W@(z+zb)=W@z+zb).
  - FFN matmuls in fp8e4 (e4m3) with DoubleRow perf mode: 256 contraction
    rows per instruction at 0.5 cyc per output element - 4x less PE time
    than f32r/bf16. Weights pre-scaled by 16 (w1) / 32 (w2) on the host to
    center fp8's [2^-6, 240] range; the 512x net factor is unwound in the
    output eviction (po * 1/512 + x2b).
  - The S x S decay matmul is block-banded (alpha^128 ~ 1.4e-6): exact
    block-diag + 1 lag matmul per 128-token block.
  - Engine balance: LN normalizes run on the otherwise-idle Pool (gpsimd)
    engine (SBUF-only ops - Pool has no PSUM port); PSUM evictions split
    between ACT and DVE; sqrt batched 4 tiles per op on ACT (single
    reciprocal_sqrt act table, no reloads).
  - FFN1 PSUM banks hold both 256-token half-chunk groups of one f-tile
    (sequential accumulation groups; PSUM data persists across a group
    start in the same bank), so each relu eviction covers [128,512].
  - Software pipelined in 512-token steps: step i+1's load/LN1/
    transpose/proj run between step i's mixing and FFN.
"""

import sys
from contextlib import ExitStack

for _p in ("/opt/trn_rl_repo", "/opt/pypackages"):
    if _p not in sys.path:
        sys.path.insert(0, _p)

import numpy as np
import ml_dtypes

import concourse.bass as bass
import concourse.mybir as mybir
import concourse.tile as tile
from concourse import bacc
from concourse.bass_utils import run_bass_kernel_spmd
from concourse.masks import make_identity

ALPHA, EPS = 0.9, 1e-5
S, B, D, FF = 2048, 16, 512, 2048
NCORES = 8
BL = B // NCORES            # batch items per core
T = 128                     # mixing block
CB = 4                      # blocks per chunk (step = 512 tokens)
NBLK = S // T               # 16
NCHUNK = NBLK // CB         # 4
NFT = FF // 128             # 16 f-tiles
KD = D // 128               # 4 d-tiles
NLAG = 1                    # decay lag blocks kept (lag>=2 < 2e-12 relative)
W1SC, W2SC = 16.0, 32.0     # fp8 weight pre-scales
OSC = 1.0 / (W1SC * W2SC)   # output unscale
RELU_DVE = frozenset({2, 5, 8, 11, 14})  # f-tiles whose relu evicts on DVE

F32 = mybir.dt.float32
BF16 = mybir.dt.bfloat16
F8 = mybir.dt.float8e4
AF = mybir.ActivationFunctionType
ALU = mybir.AluOpType
DR = mybir.MatmulPerfMode.DoubleRow

NPBF16 = ml_dtypes.bfloat16
NPF8 = mybir.dt.np(F8)      # ml_dtypes.float8_e4m3 (max 240)


def _host_consts():
    """Decay-matrix derived constants, f64 -> f32 (mirrors reference)."""
    i = np.arange(S, dtype=np.float64)
    diff = i[:, None] - i[None, :]
    with np.errstate(under="ignore"):
        W = np.where(diff >= 0, ALPHA ** (diff + 1), 0.0)
        W = W + np.diag(1.0 - W.sum(axis=1))
        W = W.astype(np.float32)
        blocks = [
            np.ascontiguousarray(W[c * T : (c + 1) * T, c * T : (c + 1) * T].T)
            for c in range(NBLK)
        ]
        uniq, idx = [], []
        for blk in blocks:
            for j, u in enumerate(uniq):
                if np.array_equal(blk, u):
                    idx.append(j)
                    break
            else:
                idx.append(len(uniq))
                uniq.append(blk)
        wblkT = np.stack(uniq)  # [NU, T, T]
        lags = []
        for l in range(1, NLAG + 1):
            L = W[l * T : (l + 1) * T, 0:T]
            lags.append(np.ascontiguousarray(L.T))
        wlagT = np.stack(lags)  # [NLAG, T, T]
    return wblkT.astype(np.float32), idx, wlagT.astype(np.float32)


_WBLKT, _BLKIDX, _WLAGT = _host_consts()
NU = _WBLKT.shape[0]

_NC_CACHE = {}


def build_nc():
    key = "v2"
    if key in _NC_CACHE:
        return _NC_CACHE[key]
    nc = bacc.Bacc()

    x_d = nc.declare_dram_parameter("x", [S, BL, D], F32, isOutput=False)
    wp_d = nc.declare_dram_parameter("wp", [D, D], BF16, isOutput=False)
    zb_d = nc.declare_dram_parameter("zb", [D], BF16, isOutput=False)
    w1_d = nc.declare_dram_parameter("w1t8", [D, FF], F8, isOutput=False)
    hb_d = nc.declare_dram_parameter("hb16", [FF], F32, isOutput=False)
    w2_d = nc.declare_dram_parameter("w2t8", [FF, D], F8, isOutput=False)
    b2_d = nc.declare_dram_parameter("b2", [D], F32, isOutput=False)
    wblk_d = nc.declare_dram_parameter("wblk", [NU, T, T], BF16, isOutput=False)
    wlag_d = nc.declare_dram_parameter("wlag", [NLAG, T, T], BF16, isOutput=False)
    out_d = nc.declare_dram_parameter("out", [S, BL, D], F32, isOutput=True)

    with tile.TileContext(nc) as tc, ExitStack() as ctx:
        pool = lambda name, bufs, **kw: ctx.enter_context(
            tc.tile_pool(name=name, bufs=bufs, **kw)
        )
        wgt = pool("wgt", 1)
        stage = pool("stage", 1)
        xin = pool("xin", 9)
        lnp = pool("ln", 4)
        yppp = pool("ypp", 2)
        xtp = pool("xt", 6)
        y2tp = pool("y2t", 2)
        zp = pool("z", 10)
        x2p = pool("x2", 5)
        x2bp = pool("x2b", 5)
        hp = pool("h", 2)
        outp = pool("outp", 3)
        psmm = pool("psmm", 5, space="PSUM")
        pstr = pool("pstr", 3, space="PSUM")

        # ---------------- one-time setup ----------------
        steps = [(b, c) for b in range(BL) for c in range(NCHUNK)]
        xpre = {}

        def preload_x(i):
            if i >= len(steps) or i in xpre:
                return
            b, c = steps[i]
            tiles = []
            for t in range(CB):
                s0 = (c * CB + t) * T
                xt = xin.tile([128, D], F32, tag="x")
                nc.sync.dma_start(xt[:], x_d.ap()[s0 : s0 + T, b, :])
                tiles.append(xt)
            xpre[i] = tiles

        preload_x(0)

        ident_f = stage.tile([128, 128], F32, tag="ident_f")
        make_identity(nc, ident_f[:])
        ident_bf = wgt.tile([128, 128], BF16, tag="ident_bf")
        nc.vector.tensor_copy(ident_bf[:], ident_f[:])
        ident_f8 = wgt.tile([128, 128], F8, tag="ident_f8")
        nc.vector.tensor_copy(ident_f8[:], ident_f[:])
        eps_t = wgt.tile([128, 1], F32, tag="eps")
        nc.vector.memset(eps_t[:], EPS)
        ones_r = wgt.tile([1, 128], BF16, tag="ones_r")
        nc.vector.memset(ones_r[:], 1.0)
        zb_r = wgt.tile([1, D], BF16, tag="zb_r")
        nc.sync.dma_start(zb_r[:], bass.AP(tensor=zb_d, offset=0, ap=[[0, 1], [1, D]]))
        b2_bc = wgt.tile([128, D], F32, tag="b2")
        nc.sync.dma_start(
            b2_bc[:], bass.AP(tensor=b2_d, offset=0, ap=[[0, 128], [1, D]])
        )
        hb_sb = wgt.tile([128, NFT], F32, tag="hb")
        nc.sync.dma_start(
            hb_sb[:], bass.AP(tensor=hb_d, offset=0, ap=[[1, 128], [128, NFT]])
        )
        wp_r = wgt.tile([128, KD, D], BF16, tag="wp")
        nc.sync.dma_start(wp_r[:], wp_d.ap().rearrange("(kd p) e -> p kd e", p=128))
        wblk_r = wgt.tile([128, NU, T], BF16, tag="wblk")
        nc.sync.dma_start(wblk_r[:], wblk_d.ap().rearrange("b j r -> j b r"))
        wlag_r = wgt.tile([128, NLAG, T], BF16, tag="wlag")
        nc.sync.dma_start(wlag_r[:], wlag_d.ap().rearrange("b j r -> j b r"))

        # ---------------- helpers ----------------
        def ln_stats(xts, tag):
            """4 tiles' LN stats -> (mvs [128,4,2], sq4 [128,4]=sqrt(v+eps))."""
            mvs = lnp.tile([128, CB, 2], F32, tag=tag + "mv")
            sq4 = lnp.tile([128, CB], F32, tag=tag + "sq")
            for t in range(CB):
                st = lnp.tile([128, 6], F32, tag=tag + "st")
                nc.vector.bn_stats(st[:], xts[t][:])
                nc.vector.bn_aggr(mvs[:, t, :], st[:])
                if t % 2 == 1:
                    nc.scalar.activation(
                        sq4[:, t - 1 : t + 1],
                        mvs[:, t - 1 : t + 1, 1:2].rearrange("p a b -> p (a b)"),
                        AF.Sqrt, bias=eps_t[:], scale=1.0,
                    )
            return mvs, sq4

        def norm_transp(xt, mvs, sq4, t, odt, ident, dest_ap, src_pat):
            """(xt - m)/sq -> odt tile (Pool), PE-transpose, ACT-evict."""
            ypp = yppp.tile([128, D], odt, tag="ypp")
            nc.gpsimd.tensor_scalar(
                out=ypp[:], in0=xt[:],
                scalar1=mvs[:, t, 0:1], scalar2=sq4[:, t : t + 1],
                op0=ALU.subtract, op1=ALU.divide,
            )
            pt = pstr.tile([128, 512], odt, tag="tr")
            for kd in range(KD):
                nc.tensor.transpose(
                    pt[:, kd * 128 : (kd + 1) * 128],
                    ypp[:, kd * 128 : (kd + 1) * 128],
                    ident[:],
                )
            nc.scalar.activation(dest_ap, src_pat(pt[:]), AF.Copy)

        # ---------------- pipeline stages ----------------
        zall = {b: [] for b in range(BL)}
        a_out, b_out = {}, {}

        def stage_a(i):
            if i >= len(steps):
                return
            preload_x(i)
            xts, xT = xpre.pop(i), []
            mvs, sq4 = ln_stats(xts, "l1")
            for t in range(CB):
                xTt = xtp.tile([128, KD, 128], BF16, tag="xT")
                norm_transp(xts[t], mvs, sq4, t, BF16, ident_bf, xTt[:],
                            lambda p: p.rearrange("p (a b) -> p a b", b=128))
                xT.append(xTt)
            a_out[i] = (xts, xT)

        def stage_b_mm(i):
            if i >= len(steps):
                return
            _, xT = a_out[i]
            pzs = []
            for t in range(CB):
                pz = psmm.tile([128, D], F32, tag="mm")
                for kd in range(KD):
                    nc.tensor.matmul(
                        pz[:], xT[t][:, kd, :], wp_r[:, kd, :],
                        start=(kd == 0), stop=(kd == KD - 1),
                    )
                pzs.append(pz)
            b_out[i] = pzs

        def stage_b_evict(i):
            if i >= len(steps):
                return
            b, c = steps[i]
            for t in range(CB):
                zt = zp.tile([128, D], BF16, tag="z")
                nc.scalar.activation(zt[:], b_out[i][t][:], AF.Copy)
                zall[b].append(zt)
            del b_out[i]

        stage_a(0)
        stage_b_mm(0)
        stage_b_evict(0)

        # big fp8 FFN weights: DMA'd in chunks interleaved with the early
        # pipeline so x loads and the first FFN aren't blocked.
        w18_r = wgt.tile([128, 2, 2, FF], F8, tag="w18")
        w28_r = wgt.tile([128, NFT // 2, 2, D], F8, tag="w28")
        w18_ap = w1_d.ap().rearrange("(kd2 i p) f -> p kd2 i f", p=128, i=2)
        w28_ap = w2_d.ap().rearrange("(kt i p) e -> p kt i e", p=128, i=2)
        wload = [
            lambda kd2=kd2: nc.sync.dma_start(
                w18_r[:, kd2, :, :], w18_ap[:, kd2, :, :]
            )
            for kd2 in range(2)
        ] + [
            lambda k4=k4: nc.sync.dma_start(
                w28_r[:, 2 * k4 : 2 * k4 + 2, :, :],
                w28_ap[:, 2 * k4 : 2 * k4 + 2, :, :],
            )
            for k4 in range(4)
        ]
        wload.reverse()
        if wload:
            wload.pop()()

        for i, (b, c) in enumerate(steps):
            xts, _ = a_out.pop(i)
            x2ts, x2bts, pms = [], [], []
            # --- mixing (banded) + zb ones-row bias matmul ---
            for t in range(CB):
                blk = c * CB + t
                nmix = 1 + min(blk, NLAG)
                pm = psmm.tile([128, D], F32, tag="mm")
                for l in range(nmix):
                    lhs = (
                        wblk_r[:, _BLKIDX[blk], :] if l == 0
                        else wlag_r[:, l - 1, :]
                    )
                    nc.tensor.matmul(
                        pm[:], lhs, zall[b][blk - l][:],
                        start=(l == 0), stop=False,
                    )
                nc.tensor.matmul(pm[:], ones_r[:], zb_r[:], start=False, stop=True)
                pms.append(pm)
            # --- prefetch next step's stage A ---
            if i + 1 < len(steps):
                stage_a(i + 1)
            for _ in range(min(2, len(wload))):
                wload.pop()()
            # --- x2 = pm + x (DVE); x2b = x2 + b2 (Pool); LN2 stats ---
            for t in range(CB):
                x2t = x2p.tile([128, D], F32, tag="x2")
                nc.vector.tensor_add(x2t[:], pms[t][:], xts[t][:])
                x2ts.append(x2t)
                x2bt = x2bp.tile([128, D], F32, tag="x2b")
                nc.gpsimd.tensor_add(x2bt[:], x2t[:], b2_bc[:])
                x2bts.append(x2bt)
            stage_b_mm(i + 1)
            for _ in range(min(2, len(wload))):
                wload.pop()()
            # --- LN2 + transpose into y2T [128, kd2, i, 512] fp8 ---
            y2T = y2tp.tile([128, 2, 2, 512], F8, tag="y2T")
            mvs2, sq42 = ln_stats(x2ts, "l2")
            for t in range(CB):
                norm_transp(
                    x2ts[t], mvs2, sq42, t, F8, ident_f8,
                    y2T[:, :, :, t * 128 : (t + 1) * 128],
                    lambda p: p.rearrange("p (a c b) -> p a c b", a=2, c=2, b=128),
                )
            for _ in range(min(2, len(wload))):
                wload.pop()()
            stage_b_evict(i + 1)
            # --- FFN1: fp8 DoubleRow, one PSUM bank per f-tile (two
            #     256-token groups), relu evict split ACT/DVE ---
            h8 = hp.tile([128, NFT // 2, 2, 512], F8, tag="h")
            for ft in range(NFT):
                ph = psmm.tile([128, 512], F32, tag="mm")
                for hh in range(2):
                    for kd2 in range(2):
                        nc.tensor.matmul(
                            ph[:, hh * 256 : (hh + 1) * 256],
                            w18_r[:, kd2, :, ft * 128 : (ft + 1) * 128],
                            y2T[:, kd2, :, hh * 256 : (hh + 1) * 256],
                            start=(kd2 == 0), stop=(kd2 == 1),
                            perf_mode=DR,
                        )
                hdst = h8[:, ft // 2, ft % 2, :]
                hbc = hb_sb[:, ft : ft + 1]
                if ft in RELU_DVE:
                    nc.vector.tensor_scalar(
                        out=hdst, in0=ph[:], scalar1=hbc, scalar2=0.0,
                        op0=ALU.add, op1=ALU.max,
                    )
                else:
                    nc.scalar.activation(hdst, ph[:], AF.Relu, bias=hbc, scale=1.0)
            # --- FFN2: fp8 DoubleRow, two 256-col groups per out tile ---
            for t in range(CB):
                s0 = (c * CB + t) * T
                po = psmm.tile([128, D], F32, tag="mm")
                for eh in range(2):
                    for kt in range(NFT // 2):
                        nc.tensor.matmul(
                            po[:, eh * 256 : (eh + 1) * 256],
                            h8[:, kt, :, t * 128 : (t + 1) * 128],
                            w28_r[:, kt, :, eh * 256 : (eh + 1) * 256],
                            start=(kt == 0), stop=(kt == NFT // 2 - 1),
                            perf_mode=DR,
                        )
                ot = outp.tile([128, D], F32, tag="o")
                nc.vector.scalar_tensor_tensor(
                    out=ot[:], in0=po[:], scalar=OSC, in1=x2bts[t][:],
                    op0=ALU.mult, op1=ALU.add,
                )
                nc.sync.dma_start(out_d.ap()[s0 : s0 + T, b, :], ot[:])

    nc.compile()
    _NC_CACHE[key] = nc
    return nc


def _prep_inputs(x, w_lin, b_lin, w1, b1, w2, b2, g1, beta1, g2, beta2):
    f32 = np.float32
    wp = np.ascontiguousarray(w_lin.T * g1[:, None]).astype(NPBF16)
    zb = (w_lin.astype(np.float64) @ beta1.astype(np.float64) + b_lin).astype(
        f32
    ).astype(NPBF16)
    w1t8 = np.ascontiguousarray(W1SC * w1.T * g2[:, None]).astype(NPF8)
    hb16 = (W1SC * (w1.astype(np.float64) @ beta2.astype(np.float64) + b1)).astype(f32)
    w2t8 = np.ascontiguousarray(W2SC * w2.T).astype(NPF8)
    shared = {
        "wp": wp,
        "zb": zb,
        "w1t8": w1t8,
        "hb16": hb16,
        "w2t8": w2t8,
        "b2": b2.astype(f32),
        "wblk": _WBLKT.astype(NPBF16),
        "wlag": _WLAGT.astype(NPBF16),
    }
    in_maps = []
    for cc in range(NCORES):
        m = dict(shared)
        m["x"] = np.ascontiguousarray(x[:, cc * BL : (cc + 1) * BL, :]).astype(f32)
        in_maps.append(m)
    return in_maps


def kernel(**inputs):
    nc = build_nc()
    in_maps = _prep_inputs(**inputs)
    res = run_bass_kernel_spmd(nc, in_maps, list(range(NCORES)))
    out = np.concatenate([r["out"] for r in res.results], axis=1)
    return out.astype(np.float32)


if __name__ == "__main__":
    rng = np.random.default_rng(0)
    demo = {
        "x": rng.standard_normal((S, B, D)).astype(np.float32),
        "w_lin": rng.standard_normal((D, D)).astype(np.float32) * D**-0.5,
        "b_lin": rng.standard_normal((D,)).astype(np.float32) * 0.01,
        "w1": rng.standard_normal((FF, D)).astype(np.float32) * D**-0.5,
        "b1": rng.standard_normal((FF,)).astype(np.float32) * 0.01,
        "w2": rng.standard_normal((D, FF)).astype(np.float32) * FF**-0.5,
        "b2": rng.standard_normal((D,)).astype(np.float32) * 0.01,
        "g1": np.ones(D, np.float32),
        "beta1": np.zeros(D, np.float32),
        "g2": np.ones(D, np.float32),
        "beta2": np.zeros(D, np.float32),
    }
    out = kernel(**demo)
    print("ok", out.shape, out.dtype)


# revision 42
# speedup vs baseline: 1.1256x; 1.0570x over previous
"""Trainium2 Bass kernel for nn_ExpSelfAttention (dense transformer block).

Math (per batch item b):
    y  = LN(x; g1, beta1);  z = y @ w_lin.T + b_lin
    attn = W @ z            (W = causal exp-decay matrix, alpha=0.9)
    x2 = x + attn
    y2 = LN(x2; g2, beta2); h = relu(y2 @ w1.T + b1)
    out = x2 + h @ w2.T + b2

Sharding: data parallel over batch (16 / 8 cores = 2 per core); weights and
the (input-independent) decay-matrix blocks replicated. No collectives.

Kernel strategy per core (mixed precision):
  - Attention path in bf16 (proj + banded mixing matmuls, PE transposes at
    1 cyc/row); residuals and LN stats in f32. b_lin folded into the mixing
    PSUM via a K=1 ones-row bias matmul (W rows sum to 1, so W@(z+zb)=W@z+zb).
  - FFN matmuls in fp8e4 (e4m3) with DoubleRow perf mode: 256 contraction
    rows per instruction at 0.5 cyc per output element - 4x less PE time
    than f32r/bf16. Weights pre-scaled by 16 (w1) / 32 (w2) on the host to
    center fp8's [2^-6, 240] range; the 512x net factor is unwound in the
    output eviction (po * 1/512 + x2b).
  - The S x S decay matmul is block-banded (alpha^128 ~ 1.4e-6): exact
    block-diag + 1 lag matmul per 128-token block.
  - Engine balance: LN normalizes run on the otherwise-idle Pool (gpsimd)
    engine (SBUF-only ops - Pool has no PSUM port); PSUM evictions split
    between ACT and DVE; sqrt batched 4 tiles per op on ACT (single
    reciprocal_sqrt act table, no reloads).
  - FFN1 PSUM banks hold both 256-token half-chunk groups of one f-tile
    (sequential accumulation groups; PSUM data persists across a group
    start in the same bank), so each relu eviction covers [128,512].
  - Software pipelined in 512-token steps: step i+1's load/LN1/
    transpose/proj run between step i's mixing and FFN.
"""

import sys
from contextlib import ExitStack

for _p in ("/opt/trn_rl_repo", "/opt/pypackages"):
    if _p not in sys.path:
        sys.path.insert(0, _p)

import numpy as np
import ml_dtypes

import concourse.bass as bass
import concourse.mybir as mybir
import concourse.tile as tile
from concourse import bacc
from concourse.bass_utils import run_bass_kernel_spmd
from concourse.masks import make_identity

ALPHA, EPS = 0.9, 1e-5
S, B, D, FF = 2048, 16, 512, 2048
NCORES = 8
BL = B // NCORES            # batch items per core
T = 128                     # mixing block
CB = 4                      # blocks per chunk (step = 512 tokens)
NBLK = S // T               # 16
NCHUNK = NBLK // CB         # 4
NFT = FF // 128             # 16 f-tiles
KD = D // 128               # 4 d-tiles
NLAG = 1                    # decay lag blocks kept (lag>=2 < 2e-12 relative)
W1SC, W2SC = 16.0, 32.0     # fp8 weight pre-scales
OSC = 1.0 / (W1SC * W2SC)   # output unscale
RELU_DVE = frozenset({1, 3, 5, 7, 9, 11, 13, 15})  # relu evicts on DVE

F32 = mybir.dt.float32
BF16 = mybir.dt.bfloat16
F8 = mybir.dt.float8e4
AF = mybir.ActivationFunctionType
ALU = mybir.AluOpType
DR = mybir.MatmulPerfMode.DoubleRow

NPBF16 = ml_dtypes.bfloat16
NPF8 = mybir.dt.np(F8)      # ml_dtypes.float8_e4m3 (max 240)


def _host_consts():
    """Decay-matrix derived constants, f64 -> f32 (mirrors reference)."""
    i = np.arange(S, dtype=np.float64)
    diff = i[:, None] - i[None, :]
    with np.errstate(under="ignore"):
        W = np.where(diff >= 0, ALPHA ** (diff + 1), 0.0)
        W = W + np.diag(1.0 - W.sum(axis=1))
        W = W.astype(np.float32)
        blocks = [
            np.ascontiguousarray(W[c * T : (c + 1) * T, c * T : (c + 1) * T].T)
            for c in range(NBLK)
        ]
        uniq, idx = [], []
        for blk in blocks:
            for j, u in enumerate(uniq):
                if np.array_equal(blk, u):
                    idx.append(j)
                    break
            else:
                idx.append(len(uniq))
                uniq.append(blk)
        wblkT = np.stack(uniq)  # [NU, T, T]
        lags = []
        for l in range(1, NLAG + 1):
            L = W[l * T : (l + 1) * T, 0:T]
            lags.append(np.ascontiguousarray(L.T))
        wlagT = np.stack(lags)  # [NLAG, T, T]
    return wblkT.astype(np.float32), idx, wlagT.astype(np.float32)


_WBLKT, _BLKIDX, _WLAGT = _host_consts()
NU = _WBLKT.shape[0]

_NC_CACHE = {}


def build_nc():
    key = "v2"
    if key in _NC_CACHE:
        return _NC_CACHE[key]
    nc = bacc.Bacc()

    x_d = nc.declare_dram_parameter("x", [S, BL, D], F32, isOutput=False)
    wp_d = nc.declare_dram_parameter("wp", [D, D], BF16, isOutput=False)
    zb_d = nc.declare_dram_parameter("zb", [D], BF16, isOutput=False)
    w1_d = nc.declare_dram_parameter("w1t8", [D, FF], F8, isOutput=False)
    hb_d = nc.declare_dram_parameter("hb16", [FF], F32, isOutput=False)
    w2_d = nc.declare_dram_parameter("w2t8", [FF, D], F8, isOutput=False)
    b2_d = nc.declare_dram_parameter("b2", [D], F32, isOutput=False)
    wblk_d = nc.declare_dram_parameter("wblk", [NU, T, T], BF16, isOutput=False)
    wlag_d = nc.declare_dram_parameter("wlag", [NLAG, T, T], BF16, isOutput=False)
    out_d = nc.declare_dram_parameter("out", [S, BL, D], F32, isOutput=True)

    with tile.TileContext(nc) as tc, ExitStack() as ctx:
        pool = lambda name, bufs, **kw: ctx.enter_context(
            tc.tile_pool(name=name, bufs=bufs, **kw)
        )
        wgt = pool("wgt", 1)
        stage = pool("stage", 1)
        xin = pool("xin", 9)
        lnp = pool("ln", 4)
        yppp = pool("ypp", 2)
        xtp = pool("xt", 6)
        y2tp = pool("y2t", 2)
        zp = pool("z", 10)
        x2p = pool("x2", 5)
        x2bp = pool("x2b", 5)
        hp = pool("h", 2)
        outp = pool("outp", 3)
        psmm = pool("psmm", 5, space="PSUM")
        pstr = pool("pstr", 3, space="PSUM")

        # ---------------- one-time setup ----------------
        steps = [(b, c) for b in range(BL) for c in range(NCHUNK)]
        xpre = {}

        def preload_x(i):
            if i >= len(steps) or i in xpre:
                return
            b, c = steps[i]
            tiles = []
            for t in range(CB):
                s0 = (c * CB + t) * T
                xt = xin.tile([128, D], F32, tag="x")
                nc.sync.dma_start(xt[:], x_d.ap()[s0 : s0 + T, b, :])
                tiles.append(xt)
            xpre[i] = tiles

        preload_x(0)

        ident_f = stage.tile([128, 128], F32, tag="ident_f")
        make_identity(nc, ident_f[:])
        ident_bf = wgt.tile([128, 128], BF16, tag="ident_bf")
        nc.vector.tensor_copy(ident_bf[:], ident_f[:])
        ident_f8 = wgt.tile([128, 128], F8, tag="ident_f8")
        nc.vector.tensor_copy(ident_f8[:], ident_f[:])
        eps_t = wgt.tile([128, 1], F32, tag="eps")
        nc.vector.memset(eps_t[:], EPS)
        ones_r = wgt.tile([1, 128], BF16, tag="ones_r")
        nc.vector.memset(ones_r[:], 1.0)
        zb_r = wgt.tile([1, D], BF16, tag="zb_r")
        nc.sync.dma_start(zb_r[:], bass.AP(tensor=zb_d, offset=0, ap=[[0, 1], [1, D]]))
        b2_bc = wgt.tile([128, D], F32, tag="b2")
        nc.sync.dma_start(
            b2_bc[:], bass.AP(tensor=b2_d, offset=0, ap=[[0, 128], [1, D]])
        )
        hb_sb = wgt.tile([128, NFT], F32, tag="hb")
        nc.sync.dma_start(
            hb_sb[:], bass.AP(tensor=hb_d, offset=0, ap=[[1, 128], [128, NFT]])
        )
        wp_r = wgt.tile([128, KD, D], BF16, tag="wp")
        nc.sync.dma_start(wp_r[:], wp_d.ap().rearrange("(kd p) e -> p kd e", p=128))
        wblk_r = wgt.tile([128, NU, T], BF16, tag="wblk")
        nc.sync.dma_start(wblk_r[:], wblk_d.ap().rearrange("b j r -> j b r"))
        wlag_r = wgt.tile([128, NLAG, T], BF16, tag="wlag")
        nc.sync.dma_start(wlag_r[:], wlag_d.ap().rearrange("b j r -> j b r"))

        # ---------------- helpers ----------------
        def ln_stats(xts, tag):
            """4 tiles' LN stats -> (mvs [128,4,2], sq4 [128,4]=sqrt(v+eps))."""
            mvs = lnp.tile([128, CB, 2], F32, tag=tag + "mv")
            sq4 = lnp.tile([128, CB], F32, tag=tag + "sq")
            for t in range(CB):
                st = lnp.tile([128, 6], F32, tag=tag + "st")
                nc.vector.bn_stats(st[:], xts[t][:])
                nc.vector.bn_aggr(mvs[:, t, :], st[:])
                if t % 2 == 1:
                    nc.scalar.activation(
                        sq4[:, t - 1 : t + 1],
                        mvs[:, t - 1 : t + 1, 1:2].rearrange("p a b -> p (a b)"),
                        AF.Sqrt, bias=eps_t[:], scale=1.0,
                    )
            return mvs, sq4

        def norm_transp(xt, mvs, sq4, t, odt, ident, dest_ap, src_pat):
            """(xt - m)/sq -> odt tile (Pool), PE-transpose, ACT-evict."""
            ypp = yppp.tile([128, D], odt, tag="ypp")
            nc.gpsimd.tensor_scalar(
                out=ypp[:], in0=xt[:],
                scalar1=mvs[:, t, 0:1], scalar2=sq4[:, t : t + 1],
                op0=ALU.subtract, op1=ALU.divide,
            )
            pt = pstr.tile([128, 512], odt, tag="tr")
            for kd in range(KD):
                nc.tensor.transpose(
                    pt[:, kd * 128 : (kd + 1) * 128],
                    ypp[:, kd * 128 : (kd + 1) * 128],
                    ident[:],
                )
            nc.scalar.activation(dest_ap, src_pat(pt[:]), AF.Copy)

        # ---------------- pipeline stages ----------------
        zall = {b: [] for b in range(BL)}
        a_out, b_out = {}, {}

        def stage_a(i):
            if i >= len(steps):
                return
            preload_x(i)
            xts, xT = xpre.pop(i), []
            mvs, sq4 = ln_stats(xts, "l1")
            for t in range(CB):
                xTt = xtp.tile([128, KD, 128], BF16, tag="xT")
                norm_transp(xts[t], mvs, sq4, t, BF16, ident_bf, xTt[:],
                            lambda p: p.rearrange("p (a b) -> p a b", b=128))
                xT.append(xTt)
            a_out[i] = (xts, xT)

        def stage_b_mm(i):
            if i >= len(steps):
                return
            _, xT = a_out[i]
            pzs = []
            for t in range(CB):
                pz = psmm.tile([128, D], F32, tag="mm")
                for kd in range(KD):
                    nc.tensor.matmul(
                        pz[:], xT[t][:, kd, :], wp_r[:, kd, :],
                        start=(kd == 0), stop=(kd == KD - 1),
                    )
                pzs.append(pz)
            b_out[i] = pzs

        def stage_b_evict(i):
            if i >= len(steps):
                return
            b, c = steps[i]
            for t in range(CB):
                zt = zp.tile([128, D], BF16, tag="z")
                nc.scalar.activation(zt[:], b_out[i][t][:], AF.Copy)
                zall[b].append(zt)
            del b_out[i]

        stage_a(0)
        stage_b_mm(0)
        stage_b_evict(0)

        # big fp8 FFN weights: DMA'd in chunks interleaved with the early
        # pipeline so x loads and the first FFN aren't blocked.
        w18_r = wgt.tile([128, 2, 2, FF], F8, tag="w18")
        w28_r = wgt.tile([128, NFT // 2, 2, D], F8, tag="w28")
        w18_ap = w1_d.ap().rearrange("(kd2 i p) f -> p kd2 i f", p=128, i=2)
        w28_ap = w2_d.ap().rearrange("(kt i p) e -> p kt i e", p=128, i=2)
        wload = [
            lambda kd2=kd2: nc.sync.dma_start(
                w18_r[:, kd2, :, :], w18_ap[:, kd2, :, :]
            )
            for kd2 in range(2)
        ] + [
            lambda k4=k4: nc.sync.dma_start(
                w28_r[:, 2 * k4 : 2 * k4 + 2, :, :],
                w28_ap[:, 2 * k4 : 2 * k4 + 2, :, :],
            )
            for k4 in range(4)
        ]
        wload.reverse()
        if wload:
            wload.pop()()

        for i, (b, c) in enumerate(steps):
            xts, _ = a_out.pop(i)
            x2ts, x2bts, pms = [], [], []
            # --- mixing (banded) + zb ones-row bias matmul ---
            for t in range(CB):
                blk = c * CB + t
                nmix = 1 + min(blk, NLAG)
                pm = psmm.tile([128, D], F32, tag="mm")
                for l in range(nmix):
                    lhs = (
                        wblk_r[:, _BLKIDX[blk], :] if l == 0
                        else wlag_r[:, l - 1, :]
                    )
                    nc.tensor.matmul(
                        pm[:], lhs, zall[b][blk - l][:],
                        start=(l == 0), stop=False,
                    )
                nc.tensor.matmul(pm[:], ones_r[:], zb_r[:], start=False, stop=True)
                pms.append(pm)
            # --- prefetch next step's stage A ---
            if i + 1 < len(steps):
                stage_a(i + 1)
            for _ in range(min(2, len(wload))):
                wload.pop()()
            # --- x2 = pm + x (DVE); x2b = x2 + b2 (Pool); LN2 stats ---
            for t in range(CB):
                x2t = x2p.tile([128, D], F32, tag="x2")
                nc.vector.tensor_add(x2t[:], pms[t][:], xts[t][:])
                x2ts.append(x2t)
                x2bt = x2bp.tile([128, D], F32, tag="x2b")
                nc.gpsimd.tensor_add(x2bt[:], x2t[:], b2_bc[:])
                x2bts.append(x2bt)
            stage_b_mm(i + 1)
            for _ in range(min(2, len(wload))):
                wload.pop()()
            # --- LN2 + transpose into y2T [128, kd2, i, 512] fp8 ---
            y2T = y2tp.tile([128, 2, 2, 512], F8, tag="y2T")
            mvs2, sq42 = ln_stats(x2ts, "l2")
            for t in range(CB):
                norm_transp(
                    x2ts[t], mvs2, sq42, t, F8, ident_f8,
                    y2T[:, :, :, t * 128 : (t + 1) * 128],
                    lambda p: p.rearrange("p (a c b) -> p a c b", a=2, c=2, b=128),
                )
            for _ in range(min(2, len(wload))):
                wload.pop()()
            stage_b_evict(i + 1)
            # --- FFN1: fp8 DoubleRow, one PSUM bank per f-tile (two
            #     256-token groups), relu evict split ACT/DVE ---
            h8 = hp.tile([128, NFT // 2, 2, 512], F8, tag="h")
            for ft in range(NFT):
                ph = psmm.tile([128, 512], F32, tag="mm")
                for hh in range(2):
                    for kd2 in range(2):
                        nc.tensor.matmul(
                            ph[:, hh * 256 : (hh + 1) * 256],
                            w18_r[:, kd2, :, ft * 128 : (ft + 1) * 128],
                            y2T[:, kd2, :, hh * 256 : (hh + 1) * 256],
                            start=(kd2 == 0), stop=(kd2 == 1),
                            perf_mode=DR,
                        )
                hdst = h8[:, ft // 2, ft % 2, :]
                hbc = hb_sb[:, ft : ft + 1]
                if ft in RELU_DVE:
                    nc.vector.tensor_scalar(
                        out=hdst, in0=ph[:], scalar1=hbc, scalar2=0.0,
                        op0=ALU.add, op1=ALU.max,
                    )
                else:
                    nc.scalar.activation(hdst, ph[:], AF.Relu, bias=hbc, scale=1.0)
            # --- FFN2: fp8 DoubleRow, two 256-col groups per out tile ---
            for t in range(CB):
                s0 = (c * CB + t) * T
                po = psmm.tile([128, D], F32, tag="mm")
                for eh in range(2):
                    for kt in range(NFT // 2):
                        nc.tensor.matmul(
                            po[:, eh * 256 : (eh + 1) * 256],
                            h8[:, kt, :, t * 128 : (t + 1) * 128],
                            w28_r[:, kt, :, eh * 256 : (eh + 1) * 256],
                            start=(kt == 0), stop=(kt == NFT // 2 - 1),
                            perf_mode=DR,
                        )
                ot = outp.tile([128, D], F32, tag="o")
                nc.vector.scalar_tensor_tensor(
                    out=ot[:], in0=po[:], scalar=OSC, in1=x2bts[t][:],
                    op0=ALU.mult, op1=ALU.add,
                )
                nc.sync.dma_start(out_d.ap()[s0 : s0 + T, b, :], ot[:])

    nc.compile()
    _NC_CACHE[key] = nc
    return nc


def _prep_inputs(x, w_lin, b_lin, w1, b1, w2, b2, g1, beta1, g2, beta2):
    f32 = np.float32
    wp = np.ascontiguousarray(w_lin.T * g1[:, None]).astype(NPBF16)
    zb = (w_lin.astype(np.float64) @ beta1.astype(np.float64) + b_lin).astype(
        f32
    ).astype(NPBF16)
    w1t8 = np.ascontiguousarray(W1SC * w1.T * g2[:, None]).astype(NPF8)
    hb16 = (W1SC * (w1.astype(np.float64) @ beta2.astype(np.float64) + b1)).astype(f32)
    w2t8 = np.ascontiguousarray(W2SC * w2.T).astype(NPF8)
    shared = {
        "wp": wp,
        "zb": zb,
        "w1t8": w1t8,
        "hb16": hb16,
        "w2t8": w2t8,
        "b2": b2.astype(f32),
        "wblk": _WBLKT.astype(NPBF16),
        "wlag": _WLAGT.astype(NPBF16),
    }
    in_maps = []
    for cc in range(NCORES):
        m = dict(shared)
        m["x"] = np.ascontiguousarray(x[:, cc * BL : (cc + 1) * BL, :]).astype(f32)
        in_maps.append(m)
    return in_maps


def kernel(**inputs):
    nc = build_nc()
    in_maps = _prep_inputs(**inputs)
    res = run_bass_kernel_spmd(nc, in_maps, list(range(NCORES)))
    out = np.concatenate([r["out"] for r in res.results], axis=1)
    return out.astype(np.float32)


if __name__ == "__main__":
    rng = np.random.default_rng(0)
    demo = {
        "x": rng.standard_normal((S, B, D)).astype(np.float32),
        "w_lin": rng.standard_normal((D, D)).astype(np.float32) * D**-0.5,
        "b_lin": rng.standard_normal((D,)).astype(np.float32) * 0.01,
        "w1": rng.standard_normal((FF, D)).astype(np.float32) * D**-0.5,
        "b1": rng.standard_normal((FF,)).astype(np.float32) * 0.01,
        "w2": rng.standard_normal((D, FF)).astype(np.float32) * FF**-0.5,
        "b2": rng.standard_normal((D,)).astype(np.float32) * 0.01,
        "g1": np.ones(D, np.float32),
        "beta1": np.zeros(D, np.float32),
        "g2": np.ones(D, np.float32),
        "beta2": np.zeros(D, np.float32),
    }
    out = kernel(**demo)
    print("ok", out.shape, out.dtype)
